# revision 2
# baseline (speedup 1.0000x reference)
"""MoE top-2 routed SwiGLU MLP on 8 Trainium2 NeuronCores.

Strategy (expert parallelism):
  - 8 experts, 8 cores: core e owns expert e's weights.
  - Host-side dispatch: gather the (unique) tokens routed to each expert,
    transpose to feature-major [H, C] (C = max token count, zero padded),
    cast to bf16.  The top-2 combine weight is folded into the up-proj
    input copy (u-path is linear), so the device output is already
    combine-weighted.
  - Device: dense SwiGLU MLP per core, everything feature-on-partition,
    tokens on the moving/free dim:
        g = Wg^T x     (accumulate over 8 H-tiles of 128)
        u = Wu^T (x*comb)
        h = silu(g) * u          [2816, C] bf16
        y = Wd^T h               [1024, C] f32
  - Host-side combine: out[tokens_e] += y_e^T  (token lists are unique
    per expert, experts summed sequentially).
"""

import os
import sys

for _p in ("/opt/trn_rl_repo",):
    if _p not in sys.path and os.path.isdir(_p):
        sys.path.insert(0, _p)

from contextlib import ExitStack

import ml_dtypes
import numpy as np

import concourse.bass as bass  # noqa: F401  (engine API comes via nc)
import concourse.tile as tile
from concourse import bacc, mybir
from concourse.bass_utils import run_bass_kernel_spmd

# Problem shape (hardcoded per task instructions).
B, S, H, I, E, TOPK = 1, 2048, 1024, 2816, 8, 2
N_CORES = 8
HT = H // 128   # 8 h-tiles
IT = I // 128   # 22 i-tiles

_BF16 = ml_dtypes.bfloat16

# Compiled-program cache keyed by (C, chunks) so repeated kernel() calls
# with the same routing shape skip rebuild/recompile.
_PROG_CACHE: dict = {}


def _chunk_sizes(C: int) -> tuple[int, ...]:
    """Split C token columns into chunks of <=512 (PSUM fp32 bank limit),
    balanced and 8-aligned (C itself must be 8-aligned)."""
    nch = -(-C // 512)
    per = -(-C // nch // 8) * 8
    sizes = []
    left = C
    for _ in range(nch):
        s = min(per, left)
        sizes.append(s)
        left -= s
    assert left == 0 and all(s > 0 for s in sizes)
    return tuple(sizes)


def _build_program(C: int, chunks: tuple[int, ...]):
    """Build + compile the per-core SPMD Bass program."""
    nc = bacc.Bacc(
        "TRN2",
        target_bir_lowering=False,
        debug=False,
        enable_asserts=False,
        num_devices=N_CORES,
    )
    bf16 = mybir.dt.bfloat16
    f32 = mybir.dt.float32

    wg_d = nc.dram_tensor("wg", [H, I], bf16, kind="ExternalInput").ap()
    wu_d = nc.dram_tensor("wu", [H, I], bf16, kind="ExternalInput").ap()
    wd_d = nc.dram_tensor("wd", [I, H], bf16, kind="ExternalInput").ap()
    xg_d = nc.dram_tensor("xg", [H, C], bf16, kind="ExternalInput").ap()
    xu_d = nc.dram_tensor("xu", [H, C], bf16, kind="ExternalInput").ap()
    y_d = nc.dram_tensor("y", [H, C], f32, kind="ExternalOutput").ap()

    # (offset, size) per token chunk
    offs = []
    o = 0
    for n in chunks:
        offs.append((o, n))
        o += n

    with ExitStack() as ctx:
        tc = ctx.enter_context(tile.TileContext(nc))
        wpool = ctx.enter_context(tc.tile_pool(name="w", bufs=1))
        xpool = ctx.enter_context(tc.tile_pool(name="x", bufs=1))
        hpool = ctx.enter_context(tc.tile_pool(name="hbuf", bufs=1))
        spool = ctx.enter_context(tc.tile_pool(name="s", bufs=2))
        ypool = ctx.enter_context(tc.tile_pool(name="yst", bufs=2))
        psum = ctx.enter_context(tc.tile_pool(name="ps", bufs=2, space="PSUM"))

        xg_sb = [xpool.tile([128, C], bf16, tag=f"xg{h}", name=f"xg{h}") for h in range(HT)]
        xu_sb = [xpool.tile([128, C], bf16, tag=f"xu{h}", name=f"xu{h}") for h in range(HT)]
        wg_sb = [wpool.tile([128, I], bf16, tag=f"wg{h}", name=f"wg{h}") for h in range(HT)]
        wu_sb = [wpool.tile([128, I], bf16, tag=f"wu{h}", name=f"wu{h}") for h in range(HT)]
        wd_sb = [wpool.tile([128, H], bf16, tag=f"wd{i}", name=f"wd{i}") for i in range(IT)]
        h_sb = [hpool.tile([128, C], bf16, tag=f"h{i}", name=f"hb{i}") for i in range(IT)]

        # Token activations first (small, needed immediately).
        for h in range(HT):
            r = slice(h * 128, (h + 1) * 128)
            nc.sync.dma_start(xg_sb[h][:], xg_d[r, :])
            nc.sync.dma_start(xu_sb[h][:], xu_d[r, :])
        # Gate/up weights in 4 column groups so compute can start after the
        # first ~1/4 of the weight stream has landed.
        wgrp = I // 4
        for g in range(4):
            cols = slice(g * wgrp, (g + 1) * wgrp)
            for h in range(HT):
                r = slice(h * 128, (h + 1) * 128)
                nc.sync.dma_start(wg_sb[h][:, cols], wg_d[r, cols])
                nc.sync.dma_start(wu_sb[h][:, cols], wu_d[r, cols])
        for i in range(IT):
            r = slice(i * 128, (i + 1) * 128)
            nc.sync.dma_start(wd_sb[i][:], wd_d[r, :])

        # Phase 1: gate/up projections + silu*mul, one i-tile at a time.
        for ic in range(IT):
            icc = slice(ic * 128, (ic + 1) * 128)
            pg = [psum.tile([128, n], f32, tag=f"pg{c}", name=f"pg{c}") for c, (_, n) in enumerate(offs)]
            pu = [psum.tile([128, n], f32, tag=f"pu{c}", name=f"pu{c}") for c, (_, n) in enumerate(offs)]
            for h in range(HT):
                lw = wg_sb[h][:, icc]
                for c, (o_, n) in enumerate(offs):
                    nc.tensor.matmul(
                        pg[c][:], lw, xg_sb[h][:, o_ : o_ + n],
                        start=(h == 0), stop=(h == HT - 1),
                    )
            for h in range(HT):
                lw = wu_sb[h][:, icc]
                for c, (o_, n) in enumerate(offs):
                    nc.tensor.matmul(
                        pu[c][:], lw, xu_sb[h][:, o_ : o_ + n],
                        start=(h == 0), stop=(h == HT - 1),
                    )
            for c, (o_, n) in enumerate(offs):
                sg = spool.tile([128, n], f32, tag=f"sg{c}", name=f"sg{c}")
                nc.scalar.activation(
                    sg[:], pg[c][:], mybir.ActivationFunctionType.Silu
                )
                nc.vector.tensor_mul(h_sb[ic][:, o_ : o_ + n], sg[:], pu[c][:])

        # Phase 2: down projection, one output h-tile at a time.
        for hc in range(HT):
            hcc = slice(hc * 128, (hc + 1) * 128)
            py = [psum.tile([128, n], f32, tag=f"pg{c}", name=f"pg{c}") for c, (_, n) in enumerate(offs)]
            for i in range(IT):
                lw = wd_sb[i][:, hcc]
                for c, (o_, n) in enumerate(offs):
                    nc.tensor.matmul(
                        py[c][:], lw, h_sb[i][:, o_ : o_ + n],
                        start=(i == 0), stop=(i == IT - 1),
                    )
            y_sb = ypool.tile([128, C], f32, tag="y", name="ysb")
            for c, (o_, n) in enumerate(offs):
                nc.vector.tensor_copy(y_sb[:, o_ : o_ + n], py[c][:])
            nc.sync.dma_start(y_d[hcc, :], y_sb[:])

    nc.compile()
    return nc


def _prepare(x, expert_indices, expert_weights, gate_proj, up_proj, down_proj):
    """Host-side dispatch.  Returns (C, chunks, in_maps, token_lists)."""
    x_flat = np.asarray(x, dtype=np.float32).reshape(-1, H)
    T = x_flat.shape[0]
    idx = np.asarray(expert_indices).reshape(T, TOPK).astype(np.int64)
    w = np.asarray(expert_weights, dtype=np.float32).reshape(T, TOPK)

    comb = np.zeros((T, E), np.float32)
    np.add.at(comb, (np.arange(T)[:, None], idx), w)
    assigned = np.zeros((T, E), bool)
    assigned[np.arange(T)[:, None], idx] = True

    token_lists = [np.nonzero(assigned[:, e])[0] for e in range(E)]
    cmax = max(len(t) for t in token_lists)
    C = max(-(-cmax // 8) * 8, 64)
    chunks = _chunk_sizes(C)

    gate = np.asarray(gate_proj, dtype=np.float32)
    up = np.asarray(up_proj, dtype=np.float32)
    down = np.asarray(down_proj, dtype=np.float32)

    in_maps = []
    for e in range(E):
        tok = token_lists[e]
        n = len(tok)
        xg = np.zeros((H, C), _BF16)
        xu = np.zeros((H, C), _BF16)
        xe = x_flat[tok]                       # [n, H] f32
        xg[:, :n] = xe.T.astype(_BF16)
        xu[:, :n] = (xe * comb[tok, e][:, None]).T.astype(_BF16)
        in_maps.append(
            {
                "wg": np.ascontiguousarray(gate[e].T).astype(_BF16),   # [H, I]
                "wu": np.ascontiguousarray(up[e].T).astype(_BF16),     # [H, I]
                "wd": np.ascontiguousarray(down[e].T).astype(_BF16),   # [I, H]
                "xg": xg,
                "xu": xu,
            }
        )
    return C, chunks, in_maps, token_lists


def kernel(x, expert_indices, expert_weights, gate_proj, up_proj, down_proj):
    C, chunks, in_maps, token_lists = _prepare(
        x, expert_indices, expert_weights, gate_proj, up_proj, down_proj
    )
    key = (C, chunks)
    if key not in _PROG_CACHE:
        _PROG_CACHE[key] = _build_program(C, chunks)
    nc = _PROG_CACHE[key]

    res = run_bass_kernel_spmd(nc, in_maps, core_ids=list(range(N_CORES)))

    T = B * S
    out_flat = np.zeros((T, H), np.float32)
    for e in range(E):
        tok = token_lists[e]
        y = res.results[e]["y"]               # [H, C] f32
        out_flat[tok] += y[:, : len(tok)].T
    return out_flat.reshape(B, S, H)
